# revision 32
# baseline (speedup 1.0000x reference)
"""Trainium2 Bass kernel for nn_LstmModel: B=512, T=256, H=512 LSTM + 2-layer FC head.

Strategy (DP-8): shard batch across 8 cores (BL=64 rows each), replicate weights.
Everything SBUF-resident, all matmuls bf16 (1 cyc/col vs 2 for fp32r on HW).

Key layout: each gate lives in a PSUM "quarter" tile [128, 256]:
  rows 0:64   = batch x gate-cols 0:256   (half A)
  rows 64:128 = batch x gate-cols 256:512 (half B)
Two gates share one 2KB PSUM bank: bank_IF = [I | F], bank_GO = [G | O].

Matmuls are emitted as col-tiled concurrent PAIRS (tile_position (0,0) and
(0,64)): the stationary hT chunk is duplicated into both 64-col halves of the
PE array (the transpose uses a [I64|I64] identity so the duplicate is free),
and the two MMs stream different W column-halves into partition rows 0:64 /
64:128 simultaneously -> full 128x128 array utilization at M=64.

Gate bias is folded into the x-part matmul with K=3 stationary [x; 1; 1] and
moving rows [w_ih; b_hi; b_lo] (hi/lo bf16 split keeps bias fp32-accurate).

Elementwise chain runs in [128, 128] halves (L = h-dims {0:128, 256:384},
R = {128:256, 384:512}) so the c -> tanh -> h -> transpose -> cast tail for L
lands early and the next step's K0/K2 matmuls start while R still drains.
"""

import sys
from contextlib import ExitStack

if "/opt/trn_rl_repo" not in sys.path:
    sys.path.insert(0, "/opt/trn_rl_repo")

import numpy as np
import ml_dtypes

import concourse.bass as bass
import concourse.tile as tile
from concourse import bacc, mybir
from concourse.bass_utils import run_bass_kernel_spmd

F32 = mybir.dt.float32
BF16 = mybir.dt.bfloat16
AF = mybir.ActivationFunctionType
BFNP = ml_dtypes.bfloat16

B, T, H, HALF, TGT = 512, 256, 512, 256, 28
NCORES = 8
BL = B // NCORES          # 64 batch rows per core
TCH = 64                  # seq steps per DMA chunk

# gate order in emission / weight layout; torch row offsets in W_hh (i,f,g,o)
GATES = ("i", "g", "f", "o")
GROW = {"i": 0, "f": 512, "g": 1024, "o": 1536}
# gate -> (bank, colhalf): bank_IF holds i (cols 0:256) + f (256:512); GO: g, o
GBANK = {"i": (0, 0), "f": (0, 1), "g": (1, 0), "o": (1, 1)}
KORDER = (0, 2, 1, 3)     # L-chunks first (hT_L ready before hT_R)

_cached = {}
DBG = None        # set to a step index to dump that step's h/c tiles


def build_program():
    nc = bacc.Bacc("TRN2", target_bir_lowering=False, debug=False,
                   num_devices=NCORES)

    d_sx = nc.dram_tensor("sx", [6, T * 128], BF16, kind="ExternalInput")
    d_whh = nc.dram_tensor("whh", [128, 8192], BF16, kind="ExternalInput")
    d_wx = nc.dram_tensor("wx", [6, 1024], BF16, kind="ExternalInput")
    d_iden = nc.dram_tensor("iden", [128, 128], BF16, kind="ExternalInput")
    d_f1w = nc.dram_tensor("f1w", [128, 1024], BF16, kind="ExternalInput")
    d_f1b = nc.dram_tensor("f1b", [2, HALF], BF16, kind="ExternalInput")
    d_f2w = nc.dram_tensor("f2w", [128, 2 * TGT], BF16, kind="ExternalInput")
    d_f2b = nc.dram_tensor("f2b", [2, TGT], BF16, kind="ExternalInput")
    d_out = nc.dram_tensor("out", [BL, TGT], F32, kind="ExternalOutput")
    d_dbg_h = d_dbg_c = d_dbg_g = None
    if DBG is not None:
        d_dbg_h = nc.dram_tensor("dbg_h", [128, 256], F32, kind="ExternalOutput")
        d_dbg_c = nc.dram_tensor("dbg_c", [128, 256], F32, kind="ExternalOutput")
        d_dbg_g = nc.dram_tensor("dbg_g", [128, 1024], F32, kind="ExternalOutput")

    with tile.TileContext(nc) as tc, ExitStack() as ctx:
        consts = ctx.enter_context(tc.tile_pool(name="consts", bufs=1))
        seqp = ctx.enter_context(tc.tile_pool(name="seqp", bufs=2))
        acts = ctx.enter_context(tc.tile_pool(name="acts", bufs=2))
        state = ctx.enter_context(tc.tile_pool(name="state", bufs=2))
        pg = ctx.enter_context(tc.tile_pool(name="pg", bufs=2, space="PSUM"))

        sb_whh = consts.tile([128, 8192], BF16)
        sb_wx = consts.tile([6, 1024], BF16)
        sb_iden = consts.tile([128, 128], BF16)
        sb_f1w = consts.tile([128, 1024], BF16)
        sb_f1b = consts.tile([2, HALF], BF16)
        sb_f2w = consts.tile([128, 2 * TGT], BF16)
        sb_f2b = consts.tile([2, TGT], BF16)
        for dst, dsrc in ((sb_whh, d_whh), (sb_wx, d_wx), (sb_iden, d_iden),
                          (sb_f1w, d_f1w), (sb_f1b, d_f1b), (sb_f2w, d_f2w),
                          (sb_f2b, d_f2b)):
            nc.sync.dma_start(dst[:], dsrc.ap())

        sx_tile = None

        def emit_x(tau):
            """x-part + bias for step tau: one tile-wide MM per gate tile.

            start=True clears has_written for the WHOLE bank, so each gate
            tile sits alone in its bank and gets a single covering MM. The
            K=6 stationary is [x;1;1;0;0;0] for rows 0:64 and [0;0;0;x;1;1]
            for rows 64:128; moving rows 0-2 / 3-5 carry the two col-halves'
            [w_ih; b_hi; b_lo]."""
            nonlocal sx_tile, banks
            if tau % TCH == 0:
                sx_tile = seqp.tile([6, TCH * 128], BF16, tag="sx")
                nc.sync.dma_start(sx_tile[:],
                                  d_sx.ap()[:, tau * 128:(tau + TCH) * 128])
            tt = tau % TCH
            banks = {g: pg.tile([128, 256], F32, tag=g, bufs=1, name=f"gt_{g}")
                     for g in GATES}
            stop = (tau == 0)   # step 0 has no h accumulation
            for gi, g in enumerate(GATES):
                nc.tensor.matmul(
                    banks[g][:],
                    sx_tile[:][:, tt * 128:(tt + 1) * 128],
                    sb_wx[:][:, gi * 256:(gi + 1) * 256],
                    start=True, stop=stop)

        banks = None
        emit_x(0)

        c_t = None
        hT_L = hT_R = None

        for t in range(T):
            gb = banks
            if t > 0:
                # recurrent matmuls, cross-bank col-tiled pairs: per K-chunk
                # interleave two gates' A/B halves so up to 4 MMs with
                # distinct moving streams + distinct PSUM banks are in flight
                for g1, g2 in (("i", "g"), ("f", "o")):
                    for k in KORDER:
                        hT = hT_L if k in (0, 2) else hT_R
                        co = 0 if k in (0, 1) else 64
                        stop = (k == 3)
                        w1 = ((GATES.index(g1) * 4 + k) * 2) * 256
                        w2 = ((GATES.index(g2) * 4 + k) * 2) * 256
                        lhs = hT[:][:, co:co + 64]
                        nc.tensor.matmul(gb[g1][:][0:64, :], lhs,
                                         sb_whh[:][:, w1:w1 + 256],
                                         start=False, stop=stop)
                        nc.tensor.matmul(gb[g2][:][64:128, :], lhs,
                                         sb_whh[:][:, w2 + 256:w2 + 512],
                                         start=False, stop=stop)
                        nc.tensor.matmul(gb[g2][:][0:64, :], lhs,
                                         sb_whh[:][:, w2:w2 + 256],
                                         start=False, stop=stop)
                        nc.tensor.matmul(gb[g1][:][64:128, :], lhs,
                                         sb_whh[:][:, w1 + 256:w1 + 512],
                                         start=False, stop=stop)

            # ---- elementwise chain (ACT + DVE)
            sI = acts.tile([128, 256], BF16, tag="sI")
            tG = acts.tile([128, 256], BF16, tag="tG")
            sF = acts.tile([128, 256], F32, tag="sF")
            sO = acts.tile([128, 256], BF16, tag="sO")
            Tc = acts.tile([128, 256], BF16, tag="Tc")
            t1 = acts.tile([128, 256], F32, tag="t1")
            t2 = acts.tile([128, 256], F32, tag="t2")
            c = state.tile([128, 256], F32, tag="c")
            h = state.tile([128, 256], BF16, tag="h")

            nc.scalar.activation(sI[:], gb["i"][:], AF.Sigmoid)
            nc.scalar.activation(tG[:], gb["g"][:], AF.Tanh)
            if t > 0:
                nc.scalar.activation(sF[:], gb["f"][:], AF.Sigmoid)
            nc.scalar.activation(sO[:], gb["o"][:], AF.Sigmoid)

            ctgt = c if t == 0 else t1
            nc.vector.tensor_mul(ctgt[:], sI[:], tG[:])
            if t > 0:
                nc.vector.tensor_mul(t2[:], sF[:], c_t[:])
                nc.vector.tensor_add(c[:], t1[:], t2[:])

            # tail in halves: L-half (h chunks 0/2) lands first so the next
            # step's K0/K2 matmuls can start while the R-half drains
            for hx in range(2):
                sl = slice(hx * 128, (hx + 1) * 128)
                nc.scalar.activation(Tc[:][:, sl], c[:][:, sl], AF.Tanh)
                nc.vector.tensor_mul(h[:][:, sl], sO[:][:, sl], Tc[:][:, sl])

            # x-part for t+1 fills PE while the chain runs
            if t + 1 < T:
                emit_x(t + 1)

            # dummy matmuls anchored to chain tiles keep the PE's HAM activity
            # window busy during the chain wait (else it re-throttles the
            # clock to 1.2 GHz every step). Moving operand = freshly written
            # chain tiles, so each dummy fires as the chain advances.
            warm = pg.tile([64, 256], F32, tag="warm", bufs=1)
            for mv in (sI[:], tG[:], sO[:],
                       c[:][:, 0:128].bitcast(BF16), Tc[:]):
                nc.tensor.matmul(warm[:], sb_iden[:][:, 0:64], mv,
                                 start=True, stop=True)

            # transposes: h[0:64, f] = cols f (chunks 0/1), h[64:128, f] = cols
            # 256+f (chunks 2/3). No stationary duplication needed: both pair
            # members read the same 64-col hT slice, tile_position places it.
            # Base-0 and base-64 stationaries must NOT share a PSUM tile
            # (mixed row-group matmuls into one tile crash the runtime).
            pT0 = pg.tile([128, 128], BF16, tag="pT0", bufs=1)   # chunks 0, 1
            pT1 = pg.tile([128, 128], BF16, tag="pT1", bufs=1)   # chunks 2, 3
            nc.tensor.transpose(pT0[:][:, 0:64], h[:][0:64, 0:128],
                                sb_iden[:][0:64, 0:64])
            nc.tensor.transpose(pT1[:][:, 0:64], h[:][64:128, 0:128],
                                sb_iden[:][64:128, 0:64])
            nc.tensor.transpose(pT0[:][:, 64:128], h[:][0:64, 128:256],
                                sb_iden[:][0:64, 0:64])
            nc.tensor.transpose(pT1[:][:, 64:128], h[:][64:128, 128:256],
                                sb_iden[:][64:128, 0:64])

            hT_L = state.tile([128, 128], BF16, tag="hTL")
            hT_R = state.tile([128, 128], BF16, tag="hTR")
            # hT_L = chunks {0, 2}, hT_R = chunks {1, 3}
            nc.vector.tensor_copy(hT_L[:][:, 0:64], pT0[:][:, 0:64])
            nc.vector.tensor_copy(hT_L[:][:, 64:128], pT1[:][:, 0:64])
            nc.vector.tensor_copy(hT_R[:][:, 0:64], pT0[:][:, 64:128])
            nc.vector.tensor_copy(hT_R[:][:, 64:128], pT1[:][:, 64:128])

            if DBG is not None and t == DBG:
                hf32 = acts.tile([128, 256], F32, tag="dbgh", bufs=1)
                nc.vector.tensor_copy(hf32[:], h[:])
                nc.sync.dma_start(d_dbg_h.ap(), hf32[:])
                nc.sync.dma_start(d_dbg_c.ap(), c[:])
                gf32 = acts.tile([128, 1024], F32, tag="dbgg", bufs=1)
                for gi_, g_ in enumerate(GATES):
                    nc.vector.tensor_copy(gf32[:][:, gi_ * 256:(gi_ + 1) * 256],
                                          gb[g_][:])
                nc.sync.dma_start(d_dbg_g.ap(), gf32[:])

            c_t = c

        # ---- FC head: hid = relu(h @ fc1_w.T + b1); out = hid @ fc2_w.T + b2
        ones_f = acts.tile([2, 64], F32, tag="onesf", bufs=2)
        ones_b = acts.tile([2, 64], BF16, tag="onesb", bufs=2)
        nc.gpsimd.memset(ones_f[:], 1.0)
        nc.vector.tensor_copy(ones_b[:], ones_f[:])
        ones = ones_b[:]
        p_hid = pg.tile([64, HALF], F32, tag="i", bufs=1)
        nc.tensor.matmul(p_hid[:], ones, sb_f1b[:], start=True, stop=False)
        # stationary: single (non-dup) hT chunk slices; chunks 0,2 in hT_L
        for k in range(4):
            hT = hT_L if k in (0, 2) else hT_R
            co = 0 if k in (0, 1) else 64
            nc.tensor.matmul(p_hid[:], hT[:][:, co:co + 64],
                             sb_f1w[:][:, k * HALF:(k + 1) * HALF],
                             start=False, stop=(k == 3))
        hid = acts.tile([64, HALF], BF16, tag="hid")
        nc.scalar.activation(hid[:], p_hid[:], AF.Relu)

        pTh = pg.tile([128, 128], BF16, tag="g", bufs=1)
        nc.tensor.transpose(pTh[:][:, 0:64], hid[:][:, 0:128],
                            sb_iden[:][0:64, 0:64])
        nc.tensor.transpose(pTh[:][:, 64:128], hid[:][:, 128:256],
                            sb_iden[:][0:64, 0:64])
        hidT = acts.tile([128, 128], BF16, tag="hidT")
        nc.vector.tensor_copy(hidT[:], pTh[:])

        p_out = pg.tile([64, TGT], F32, tag="f", bufs=1)
        nc.tensor.matmul(p_out[:], ones, sb_f2b[:], start=True, stop=False)
        for k in range(2):
            nc.tensor.matmul(p_out[:], hidT[:][:, k * 64:(k + 1) * 64],
                             sb_f2w[:][:, k * TGT:(k + 1) * TGT],
                             start=False, stop=(k == 1))
        res = acts.tile([BL, TGT], F32, tag="res")
        nc.vector.tensor_copy(res[:], p_out[:])
        nc.sync.dma_start(d_out.ap(), res[:])

    if not nc.is_finalized():
        nc.finalize()
    return nc


def _bf(x):
    return np.asarray(x, np.float32).astype(BFNP)


def _prep_shared(W_ih, W_hh, b_ih, b_hh, fc1_w, fc1_b, fc2_w, fc2_b):
    W_hh = np.asarray(W_hh, np.float32)
    wih = np.asarray(W_ih, np.float32)[:, 0]
    bias = np.asarray(b_ih, np.float32) + np.asarray(b_hh, np.float32)
    b_hi = bias.astype(BFNP).astype(np.float32)
    b_lo = bias - b_hi

    whh = np.empty((128, 8192), BFNP)
    for gi, g in enumerate(GATES):
        for k in range(4):
            for hf in range(2):
                off = ((gi * 4 + k) * 2 + hf) * 256
                rows = GROW[g] + hf * 256
                whh[:, off:off + 256] = _bf(
                    W_hh[rows:rows + 256, k * 128:(k + 1) * 128].T)

    wx = np.empty((6, 1024), BFNP)
    for gi, g in enumerate(GATES):
        for hf in range(2):            # hf -> moving row group (out row half)
            off = gi * 256
            rows = slice(GROW[g] + hf * 256, GROW[g] + hf * 256 + 256)
            wx[hf * 3 + 0, off:off + 256] = _bf(wih[rows])
            wx[hf * 3 + 1, off:off + 256] = _bf(b_hi[rows])
            wx[hf * 3 + 2, off:off + 256] = _bf(b_lo[rows])

    iden = np.zeros((128, 128), np.float32)
    ii = np.arange(128)
    iden[ii[:, None] % 64 == ii[None, :] % 64] = 1.0

    f1w = np.empty((128, 1024), BFNP)
    for k in range(4):
        f1w[:, k * HALF:(k + 1) * HALF] = _bf(
            np.asarray(fc1_w, np.float32)[:, k * 128:(k + 1) * 128].T)
    b1 = np.asarray(fc1_b, np.float32)
    b1_hi = b1.astype(BFNP).astype(np.float32)
    f1b = np.stack([b1_hi, b1 - b1_hi]).astype(BFNP)

    f2w = np.empty((128, 2 * TGT), BFNP)
    for k in range(2):
        f2w[:, k * TGT:(k + 1) * TGT] = _bf(
            np.asarray(fc2_w, np.float32)[:, k * 128:(k + 1) * 128].T)
    b2 = np.asarray(fc2_b, np.float32)
    b2_hi = b2.astype(BFNP).astype(np.float32)
    f2b = np.stack([b2_hi, b2 - b2_hi]).astype(BFNP)

    return {"whh": whh, "wx": wx, "iden": iden.astype(BFNP),
            "f1w": f1w, "f1b": f1b, "f2w": f2w, "f2b": f2b}


def run(inputs, trace=False):
    if "nc" not in _cached:
        _cached["nc"] = build_program()
    nc = _cached["nc"]
    shared = _prep_shared(
        inputs["W_ih"], inputs["W_hh"], inputs["b_ih"], inputs["b_hh"],
        inputs["fc1_w"], inputs["fc1_b"], inputs["fc2_w"], inputs["fc2_b"])
    seq = np.asarray(inputs["sequence"], np.float32)[:, :, 0]   # [B, T]
    in_maps = []
    for cid in range(NCORES):
        xs = seq[cid * BL:(cid + 1) * BL, :].T                  # [T, 64]
        z = np.zeros_like(xs)
        o = np.ones_like(xs)
        # per step block of 128 cols: rows 0-2 = [x;1;1 | 0], rows 3-5 = [0 | x;1;1]
        sx = np.stack([
            np.concatenate([xs, z], 1), np.concatenate([o, z], 1),
            np.concatenate([o, z], 1), np.concatenate([z, xs], 1),
            np.concatenate([z, o], 1), np.concatenate([z, o], 1),
        ]).reshape(6, T * 128).astype(BFNP)
        in_maps.append({"sx": sx, **shared})
    br = run_bass_kernel_spmd(nc, in_maps, list(range(NCORES)), trace=trace)
    out = np.concatenate([br.results[i]["out"] for i in range(NCORES)], axis=0)
    return out[:, :, None].astype(np.float32), br


def kernel(**inputs):
    out, _ = run(inputs)
    return out
